# revision 25
# baseline (speedup 1.0000x reference)
"""Trainium2 Bass kernel for the CIFAR10 Monarch MLP (7 monarch layers + log_softmax).

Strategy
--------
Pure data parallel over 8 NeuronCores: each core takes a 1024-row batch shard;
the ~9M-param block-diagonal weights are replicated (bf16 on device).

On-device dataflow is feature-major: activations live in SBUF as
[features (128-partition tiles), batch (free dim)], fully SBUF-resident across
all layers; only x, the weights and the final log-probs cross HBM.

Layers 1-3 keep the monarch two-GEMM structure expressed as block-sparse
matmuls over the *effective* weight matrices (butterfly permutation folded
into W1 on the host).  Layers 4-6 are fused into a single dense GEMM each
(W1eff @ W2eff), which has FEWER 128x128 tiles than the factored form at
these sizes and drops the mid evictions.  Layer 7 is fused and computed
batch-major (activation tile as the stationary matmul operand), so the
logits land as [batch, 10] in PSUM and log_softmax needs no transposes.

log_softmax avoids the Ln activation function entirely: ln(S) is computed
with a DVE exponent-extraction estimate refined by one Newton step that only
needs Exp.  Every activation function used (Copy / Relu / Exp) lives in one
activation-table set, so the program loads act tables exactly once - extra
InstLoadActFuncSet switches measurably slow *all* PE matmuls by ~20%.
"""

import os as _os

import numpy as np

import concourse.bacc as bacc_mod
import concourse.mybir as mybir
import concourse.tile as tile
from concourse.bass_utils import run_bass_kernel_spmd

# ----------------------------------------------------------------- problem dims
BATCH = 8192
IN_FEATURES = 3072
NCORES = 8
BPC = BATCH // NCORES          # 1024 batch rows per core
NOUT = 10

SHAPES = [((4, 750, 768), (4, 750, 750)),
          ((4, 500, 750), (4, 500, 500)),
          ((4, 250, 500), (4, 250, 250)),
          ((4, 125, 250), (4, 125, 125)),
          ((4, 50, 125), (4, 50, 50)),
          ((4, 25, 50), (4, 25, 25)),
          ((4, 3, 25), (4, 3, 3))]
NLAYERS = 7
NMONARCH = 3                   # layers 0..2 stay factored; 3..5 dense; 6 form-B

F32 = mybir.dt.float32
I32 = mybir.dt.int32
ACT_DT_NAME = _os.environ.get("KERNEL_MM_DT", "bf16")
ACT_DT = {"fp32": mybir.dt.float32,
          "fp32r": mybir.dt.float32r,
          "bf16": mybir.dt.bfloat16}[ACT_DT_NAME]

LN2_OVER_2P23 = float(np.log(2.0) / (1 << 23))   # 8.262958e-08
EXP_BITS_BIAS = 1065353216.0                     # bitcast(1.0f)


# ------------------------------------------------------------------ layouts
class Layout:
    """Placement of 4 feature blocks of size Sb into 128-partition tiles."""

    @classmethod
    def from_positions(cls, Sb, ntiles, feat_tile, feat_row):
        self = object.__new__(cls)
        self.Sb = Sb
        self.ntiles = ntiles
        self.feat_tile = feat_tile
        self.feat_row = feat_row
        self._finish()
        return self

    def _finish(self):
        self.valid = np.zeros(self.ntiles, np.int64)
        for k in range(4):
            for t, r in zip(self.feat_tile[k], self.feat_row[k]):
                self.valid[t] = max(self.valid[t], r + 1)
        self.grow = [self.feat_tile[k] * 128 + self.feat_row[k]
                     for k in range(4)]
        self.tiles_of_block = [sorted(set(self.feat_tile[k].tolist()))
                               for k in range(4)]


def simple_layout(Sb):
    """Blocks >= 128 rows: chunked over dedicated tiles. 65..127: one tile
    each. <= 64: packed at 32-aligned offsets."""
    if Sb >= 128:
        cpb = (Sb + 127) // 128
        ft, fr = [], []
        for k in range(4):
            i = np.arange(Sb)
            ft.append(k * cpb + i // 128)
            fr.append(i % 128)
        return Layout.from_positions(Sb, 4 * cpb, ft, fr)
    stride = ((Sb + 31) // 32) * 32
    bpt = max(1, 128 // stride)
    ntiles = (4 + bpt - 1) // bpt
    ft, fr = [], []
    for k in range(4):
        i = np.arange(Sb)
        ft.append(np.full(Sb, k // bpt, np.int64))
        fr.append((k % bpt) * stride + i)
    return Layout.from_positions(Sb, ntiles, ft, fr)


def grouped_mid_layout(R, Q):
    """Mid layout with features regrouped by input block k (R >= 125)."""
    cpb = max(1, (R + 127) // 128)
    block_rows = cpb * 128
    Gp = block_rows // 4
    ft, fr = [], []
    for l in range(4):
        rs = np.arange(R)
        ks = (4 * rs + l) // Q
        pos = np.empty(R, np.int64)
        for k in range(4):
            idx = rs[ks == k]
            assert len(idx) <= Gp
            pos[idx] = k * Gp + np.arange(len(idx))
        ft.append(l * cpb + pos // 128)
        fr.append(pos % 128)
    return Layout.from_positions(R, 4 * cpb, ft, fr)


def build_full_mats(w1_shape, w2_shape, lin, lmid, lout):
    """Shapes of the dense effective matrices (values filled on the host)."""
    return ((lin.ntiles * 128, lmid.ntiles * 128),
            (lmid.ntiles * 128, lout.ntiles * 128))


class LayerPlan:
    """Factored (monarch) layer: two block-sparse GEMMs."""

    def __init__(self, li, w1_shape, w2_shape, in_layout):
        _, Q, P = w1_shape
        _, S, R = w2_shape
        self.li, self.P, self.Q, self.R, self.S = li, P, Q, R, S
        self.lin = in_layout
        self.lmid = grouped_mid_layout(R, Q) if R >= 125 else simple_layout(R)
        self.lout = simple_layout(S)
        self.ngroups = 2 if li == 0 else 1
        self._build()

    def _build(self):
        Q, R, S = self.Q, self.R, self.S
        ks_of = [(4 * np.arange(R) + l) // Q for l in range(4)]

        need1 = {}
        for l in range(4):
            for r in range(R):
                mt = int(self.lmid.feat_tile[l][r])
                k = int(ks_of[l][r])
                need1.setdefault(mt, set()).update(self.lin.tiles_of_block[k])
        self.g1_chains = {mt: sorted(its) for mt, its in need1.items()}
        self.w1_blocks = [(mt, it) for mt in sorted(need1)
                          for it in self.g1_chains[mt]]
        self.w1_block_of = {p: i for i, p in enumerate(self.w1_blocks)}

        need2 = {}
        for l in range(4):
            for s in range(S):
                ot = int(self.lout.feat_tile[l][s])
                need2.setdefault(ot, set()).update(self.lmid.tiles_of_block[l])
        self.g2_chains = {ot: sorted(mts) for ot, mts in need2.items()}
        self.w2_blocks = [(ot, mt) for ot in sorted(need2)
                          for mt in self.g2_chains[ot]]
        self.w2_block_of = {p: i for i, p in enumerate(self.w2_blocks)}

        self.mid_tiles_of_l = [self.lmid.tiles_of_block[l] for l in range(4)]
        self.out_tiles_of_l = [self.lout.tiles_of_block[l] for l in range(4)]

    def group_lset(self, g):
        return range(4) if self.ngroups == 1 else range(2 * g, 2 * g + 2)

    def group_w1range(self, g):
        mts = {t for l in self.group_lset(g) for t in self.mid_tiles_of_l[l]}
        idxs = [i for i, (mt, _) in enumerate(self.w1_blocks) if mt in mts]
        assert idxs == list(range(idxs[0], idxs[0] + len(idxs)))
        return idxs[0], len(idxs)

    def group_w2range(self, g):
        ots = {t for l in self.group_lset(g) for t in self.out_tiles_of_l[l]}
        idxs = [i for i, (ot, _) in enumerate(self.w2_blocks) if ot in ots]
        assert idxs == list(range(idxs[0], idxs[0] + len(idxs)))
        return idxs[0], len(idxs)

    def full_mats(self, w1, w2):
        """Dense effective matrices (butterfly folded into W1)."""
        Q, R = self.Q, self.R
        W1full = np.zeros((self.lin.ntiles * 128, self.lmid.ntiles * 128),
                          np.float32)
        W2full = np.zeros((self.lmid.ntiles * 128, self.lout.ntiles * 128),
                          np.float32)
        for l in range(4):
            js = 4 * np.arange(R) + l
            ks, qs = js // Q, js % Q
            mcols = self.lmid.grow[l]
            for k in range(4):
                sel = np.where(ks == k)[0]
                if len(sel) == 0:
                    continue
                W1full[np.ix_(self.lin.grow[k], mcols[sel])] = \
                    np.ascontiguousarray(w1[k, qs[sel], :].T)
            W2full[np.ix_(self.lmid.grow[l], self.lout.grow[l])] = \
                np.ascontiguousarray(w2[l].T)
        return W1full, W2full

    def build_weights(self, w1, w2):
        """Host: gather the nonzero 128x128 tiles into [128, nblocks*128]."""
        W1full, W2full = self.full_mats(w1, w2)
        W1m = np.zeros((128, 128 * len(self.w1_blocks)), np.float32)
        for i, (mt, it) in enumerate(self.w1_blocks):
            W1m[:, i * 128:(i + 1) * 128] = \
                W1full[it * 128:(it + 1) * 128, mt * 128:(mt + 1) * 128]
        W2m = np.zeros((128, 128 * len(self.w2_blocks)), np.float32)
        for i, (ot, mt) in enumerate(self.w2_blocks):
            W2m[:, i * 128:(i + 1) * 128] = \
                W2full[mt * 128:(mt + 1) * 128, ot * 128:(ot + 1) * 128]
        return W1m, W2m


class DensePlan:
    """Fused layer: one dense GEMM over the product W1eff @ W2eff."""

    def __init__(self, li, w1_shape, w2_shape, in_layout):
        self.li = li
        self.fact = LayerPlan(li, w1_shape, w2_shape, in_layout)
        self.lin = in_layout
        self.lout = self.fact.lout
        self.blocks = [(ot, it)
                       for ot in range(self.lout.ntiles)
                       for it in range(self.lin.ntiles)]
        self.block_of = {p: i for i, p in enumerate(self.blocks)}

    def build_weights(self, w1, w2):
        W1full, W2full = self.fact.full_mats(w1, w2)
        Wd = W1full @ W2full
        Wm = np.zeros((128, 128 * len(self.blocks)), np.float32)
        for i, (ot, it) in enumerate(self.blocks):
            Wm[:, i * 128:(i + 1) * 128] = \
                Wd[it * 128:(it + 1) * 128, ot * 128:(ot + 1) * 128]
        return Wm


class FormBPlan:
    """Final layer: fused dense [in_rows x NOUT], computed batch-major with
    the activation tile as the stationary operand."""

    def __init__(self, li, w1_shape, w2_shape, in_layout):
        self.li = li
        self.fact = LayerPlan(li, w1_shape, w2_shape, in_layout)
        self.lin = in_layout
        assert self.lin.ntiles == 1
        self.in_valid = int(self.lin.valid[0])

    def build_weights(self, w1, w2):
        W1full, W2full = self.fact.full_mats(w1, w2)
        Wd = W1full @ W2full                       # [in_rows, out_grow cols]
        lout = self.fact.lout
        cols = [int(lout.grow[n // 3][n % 3]) for n in range(NOUT)]
        W = np.zeros((128, NOUT), np.float32)
        W[:self.in_valid + 0, :] = Wd[:self.in_valid, cols][: 128]
        return W


def build_plans():
    plans = []
    lin = simple_layout(SHAPES[0][0][2])
    for i, (s1, s2) in enumerate(SHAPES):
        if i < NMONARCH:
            pl = LayerPlan(i, s1, s2, lin)
        elif i < NLAYERS - 1:
            pl = DensePlan(i, s1, s2, lin)
        else:
            pl = FormBPlan(i, s1, s2, lin)
        plans.append(pl)
        lin = pl.lout if i < NLAYERS - 1 else None
    return plans


# --------------------------------------------------- numpy model of the schedule
def numpy_forward(plans, weights, xT):
    B = xT.shape[1]
    h = np.zeros((plans[0].lin.ntiles * 128, B), np.float32)
    h[:xT.shape[0]] = xT
    for pl in plans:
        if isinstance(pl, LayerPlan):
            W1m, W2m = weights[pl.li]
            mid = np.zeros((pl.lmid.ntiles * 128, B), np.float32)
            for mt, its in pl.g1_chains.items():
                V = pl.lmid.valid[mt]
                acc = np.zeros((V, B), np.float32)
                for it in its:
                    ln = pl.lin.valid[it]
                    b = pl.w1_block_of[(mt, it)]
                    acc += W1m[0:ln, b * 128:b * 128 + V].T @ \
                        h[it * 128: it * 128 + ln]
                mid[mt * 128: mt * 128 + V] = acc
            out = np.zeros((pl.lout.ntiles * 128, B), np.float32)
            for ot, mts in pl.g2_chains.items():
                V = pl.lout.valid[ot]
                acc = np.zeros((V, B), np.float32)
                for mt in mts:
                    ln = pl.lmid.valid[mt]
                    b = pl.w2_block_of[(ot, mt)]
                    acc += W2m[0:ln, b * 128:b * 128 + V].T @ \
                        mid[mt * 128: mt * 128 + ln]
                out[ot * 128: ot * 128 + V] = acc
            h = np.maximum(out, 0.0)
        elif isinstance(pl, DensePlan):
            Wm = weights[pl.li]
            out = np.zeros((pl.lout.ntiles * 128, B), np.float32)
            for ot in range(pl.lout.ntiles):
                V = pl.lout.valid[ot]
                acc = np.zeros((V, B), np.float32)
                for it in range(pl.lin.ntiles):
                    ln = pl.lin.valid[it]
                    b = pl.block_of[(ot, it)]
                    acc += Wm[0:ln, b * 128:b * 128 + V].T @ \
                        h[it * 128: it * 128 + ln]
                out[ot * 128: ot * 128 + V] = acc
            h = np.maximum(out, 0.0)
        else:
            W = weights[pl.li]                     # [128, NOUT]
            ln = pl.in_valid
            logits = h[0:ln, :].T @ W[0:ln, :]     # [B, NOUT]
            t = logits
            s = np.exp(t).sum(axis=1, keepdims=True)
            return t - np.log(s)
    raise AssertionError


# ------------------------------------------------------------------ bass program
def build_program(plans):
    nc = bacc_mod.Bacc()

    xT = nc.dram_tensor("xT", [plans[0].lin.ntiles, 128, BPC], ACT_DT,
                        kind="ExternalInput")
    wdram = []
    for i, p in enumerate(plans):
        if isinstance(p, LayerPlan):
            wdram.append((
                nc.dram_tensor(f"w1c_{i}", [128, 128 * len(p.w1_blocks)],
                               ACT_DT, kind="ExternalInput"),
                nc.dram_tensor(f"w2c_{i}", [128, 128 * len(p.w2_blocks)],
                               ACT_DT, kind="ExternalInput")))
        elif isinstance(p, DensePlan):
            wdram.append(nc.dram_tensor(f"wdc_{i}", [128, 128 * len(p.blocks)],
                                        ACT_DT, kind="ExternalInput"))
        else:
            wdram.append(nc.dram_tensor(f"w7c_{i}", [128, NOUT], ACT_DT,
                                        kind="ExternalInput"))
    y = nc.dram_tensor("y", [BPC, NOUT], F32, kind="ExternalOutput")

    with tile.TileContext(nc) as tc:
        with (
            tc.tile_pool(name="sb", bufs=1) as sb,
            tc.tile_pool(name="ps", bufs=1, space="PSUM") as ps,
        ):
            evict_flip = [0]

            def evict(dst_ap, src_ap, relu):
                e = evict_flip[0] = evict_flip[0] ^ 1
                if relu:
                    if e:
                        nc.vector.tensor_scalar_max(dst_ap, src_ap, 0.0)
                    else:
                        nc.scalar.activation(dst_ap, src_ap,
                                             mybir.ActivationFunctionType.Relu)
                else:
                    if e:
                        nc.vector.tensor_copy(dst_ap, src_ap)
                    else:
                        nc.scalar.copy(dst_ap, src_ap)

            # ---- PE p-state warm-up: the tensor engine needs ~3us of
            # continuous work to reach max clock, and the first real chains
            # otherwise run 2x slow while DMA still streams x/weights.  Burn
            # the idle startup window with dummy matmuls on a zeroed scratch
            # tile (results discarded).
            scr = sb.tile([128, 512], ACT_DT, name="scr", tag="scr")
            nc.vector.memset(scr[:, :], 0.0)
            pwarm = ps.tile([128, 512], F32, name="pwarm", tag="p7b", bufs=2)
            for _ in range(13):
                nc.tensor.matmul(pwarm[:, :], scr[0:128, 0:128], scr[:, :],
                                 start=True, stop=True)

            # ---- startup: first weight slices before/interleaved with x ----
            pl0 = plans[0]
            b1_0, b1_n = pl0.group_w1range(0)
            b2_0, b2_n = pl0.group_w2range(0)
            w1sb0 = sb.tile([128, b1_n * 128], ACT_DT, name="w1sb_0_0",
                            tag="w1")
            w2sb0 = sb.tile([128, b2_n * 128], ACT_DT, name="w2sb_0_0",
                            tag="w2")
            hin = sb.tile([128, pl0.lin.ntiles, BPC], ACT_DT,
                          name="h_in0", tag="hA")

            w1d0, w2d0 = wdram[0]
            # G1 chains of group 0 are emitted interleaved across l=0,1 (see
            # below); ship weight slices in that order, interleaved with x.
            g0_mts = []
            for a, b in zip(pl0.mid_tiles_of_l[0], pl0.mid_tiles_of_l[1]):
                g0_mts.extend((a, b))
            w1_order = []        # (block_start, block_count) per chain
            for mt in g0_mts:
                idxs = [pl0.w1_block_of[(mt, it)] - b1_0
                        for it in pl0.g1_chains[mt]]
                w1_order.append((min(idxs), len(idxs)))
            xq = [(0, 2), (2, 4), (4, 6), (6, 9), (9, 12), (12, 16),
                  (16, 20), (20, 24)]
            xi = 0

            def ship_x(n=1):
                nonlocal xi
                for _ in range(n):
                    if xi < len(xq):
                        t0, t1 = xq[xi]
                        xi += 1
                        nc.sync.dma_start(
                            out=hin[:, t0:t1, :],
                            in_=xT[t0:t1].rearrange("t p n -> p t n"))

            for ci, (s0, ns) in enumerate(w1_order):
                nc.sync.dma_start(
                    out=w1sb0[:, s0 * 128:(s0 + ns) * 128],
                    in_=w1d0[:, (b1_0 + s0) * 128:(b1_0 + s0 + ns) * 128])
                if ci == 0:
                    ship_x(2)
                elif ci % 2 == 1:
                    ship_x()
            ship_x(2)
            h2 = b2_n // 2
            nc.sync.dma_start(out=w2sb0[:, 0:h2 * 128],
                              in_=w2d0[:, b2_0 * 128:(b2_0 + h2) * 128])
            ship_x(len(xq))
            nc.sync.dma_start(out=w2sb0[:, h2 * 128:b2_n * 128],
                              in_=w2d0[:, (b2_0 + h2) * 128:(b2_0 + b2_n) * 128])

            # ---- monarch layers 0..NMONARCH-1 ----
            for li in range(NMONARCH):
                pl = plans[li]

                hnext = sb.tile([128, pl.lout.ntiles, BPC], ACT_DT,
                                name=f"h_{li + 1}",
                                tag="hB" if li % 2 == 0 else "hA")

                def g1_tile(mt, mtloc, midl, w1sb, b0, pl=pl, hin=hin):
                    V = int(pl.lmid.valid[mt])
                    its = pl.g1_chains[mt]
                    for cs in range(2):
                        c0 = cs * 512
                        pm = ps.tile([128, 512], F32, name=f"pm_{pl.li}",
                                     tag="pmid", bufs=3)
                        for j, it in enumerate(its):
                            ln = int(pl.lin.valid[it])
                            b = pl.w1_block_of[(mt, it)] - b0
                            nc.tensor.matmul(pm[0:V, :],
                                             w1sb[0:ln, b * 128:b * 128 + V],
                                             hin[0:ln, it, c0:c0 + 512],
                                             start=(j == 0),
                                             stop=(j == len(its) - 1))
                        evict(midl[0:V, mtloc, c0:c0 + 512], pm[0:V, :],
                              relu=False)

                def g2_tile(ot, mid_of, w2sb, b0, pl=pl, hnext=hnext):
                    V = int(pl.lout.valid[ot])
                    mts = pl.g2_chains[ot]
                    for cs in range(2):
                        c0 = cs * 512
                        po = ps.tile([128, 512], F32, name=f"po_{pl.li}",
                                     tag="pout", bufs=3)
                        for j, mt in enumerate(mts):
                            ln = int(pl.lmid.valid[mt])
                            b = pl.w2_block_of[(ot, mt)] - b0
                            midl, loc = mid_of[mt]
                            nc.tensor.matmul(po[0:V, :],
                                             w2sb[0:ln, b * 128:b * 128 + V],
                                             midl[0:ln, loc, c0:c0 + 512],
                                             start=(j == 0),
                                             stop=(j == len(mts) - 1))
                        evict(hnext[0:V, ot, c0:c0 + 512], po[0:V, :],
                              relu=True)

                for g in range(pl.ngroups):
                    ls = list(pl.group_lset(g))
                    b1_0, b1_n = pl.group_w1range(g)
                    b2_0, b2_n = pl.group_w2range(g)

                    if li == 0 and g == 0:
                        w1sb, w2sb = w1sb0, w2sb0
                    else:
                        w1d, w2d = wdram[li]
                        w1sb = sb.tile([128, b1_n * 128], ACT_DT,
                                       name=f"w1sb_{li}_{g}", tag="w1")
                        nc.sync.dma_start(
                            out=w1sb[:, :],
                            in_=w1d[:, b1_0 * 128:(b1_0 + b1_n) * 128])
                        w2sb = sb.tile([128, b2_n * 128], ACT_DT,
                                       name=f"w2sb_{li}_{g}", tag="w2")
                        nc.sync.dma_start(
                            out=w2sb[:, :],
                            in_=w2d[:, b2_0 * 128:(b2_0 + b2_n) * 128])

                    if li == 0 and g == 0:
                        # interleave G1 chains across the two l's so the
                        # earliest-arriving x tiles feed as much work as
                        # possible; G2 for both l's afterwards.
                        mid_of = {}
                        midls = {}
                        for l in ls:
                            mts_l = pl.mid_tiles_of_l[l]
                            midls[l] = sb.tile([128, len(mts_l), BPC], ACT_DT,
                                               name=f"mid_{li}_{l}",
                                               tag="midb", bufs=2)
                            for loc, mt in enumerate(mts_l):
                                mid_of[mt] = (midls[l], loc)
                        for mt in g0_mts:
                            midl, loc = mid_of[mt]
                            g1_tile(mt, loc, midl, w1sb, b1_0)
                        for l in ls:
                            for ot in pl.out_tiles_of_l[l]:
                                g2_tile(ot, mid_of, w2sb, b2_0)
                    else:
                        # per-l pipeline with one-block lookahead
                        mid_of = {}
                        pend = None
                        for l in ls:
                            mts_l = pl.mid_tiles_of_l[l]
                            midl = sb.tile([128, len(mts_l), BPC], ACT_DT,
                                           name=f"mid_{li}_{l}", tag="midb",
                                           bufs=2)
                            for loc, mt in enumerate(mts_l):
                                mid_of[mt] = (midl, loc)
                                g1_tile(mt, loc, midl, w1sb, b1_0)
                            if pend is not None:
                                for ot in pl.out_tiles_of_l[pend]:
                                    g2_tile(ot, mid_of, w2sb, b2_0)
                            pend = l
                        for ot in pl.out_tiles_of_l[pend]:
                            g2_tile(ot, mid_of, w2sb, b2_0)

                hin = hnext

            # ---- dense fused layers ----
            for li in range(NMONARCH, NLAYERS - 1):
                pl = plans[li]
                hnext = sb.tile([128, pl.lout.ntiles, BPC], ACT_DT,
                                name=f"h_{li + 1}",
                                tag="hB" if li % 2 == 0 else "hA")
                wsb = sb.tile([128, 128 * len(pl.blocks)], ACT_DT,
                              name=f"wd_{li}", tag="wd", bufs=2)
                nc.sync.dma_start(out=wsb[:, :], in_=wdram[li][:, :])
                for ot in range(pl.lout.ntiles):
                    V = int(pl.lout.valid[ot])
                    for cs in range(2):
                        c0 = cs * 512
                        po = ps.tile([128, 512], F32, name=f"po_{li}",
                                     tag="pout", bufs=3)
                        for j, it in enumerate(range(pl.lin.ntiles)):
                            ln = int(pl.lin.valid[it])
                            b = pl.block_of[(ot, it)]
                            nc.tensor.matmul(po[0:V, :],
                                             wsb[0:ln, b * 128:b * 128 + V],
                                             hin[0:ln, it, c0:c0 + 512],
                                             start=(j == 0),
                                             stop=(j == pl.lin.ntiles - 1))
                        evict(hnext[0:V, ot, c0:c0 + 512], po[0:V, :],
                              relu=True)
                hin = hnext

            # ---- final layer: batch-major logits + log_softmax ----
            pl = plans[NLAYERS - 1]
            ln = pl.in_valid
            w7sb = sb.tile([128, NOUT], ACT_DT, name="w7", tag="wd", bufs=2)
            nc.sync.dma_start(out=w7sb[:, :], in_=wdram[NLAYERS - 1][:, :])

            nch = BPC // 128
            logit = sb.tile([128, nch, NOUT], F32, name="logit", tag="logit")
            esb = sb.tile([128, nch, NOUT], F32, name="esb", tag="esb")
            esum = sb.tile([128, nch], F32, name="esum", tag="esum")
            for ch in range(nch):
                po = ps.tile([128, NOUT], F32, name="po7", tag="p7b", bufs=2)
                nc.tensor.matmul(po[:, :],
                                 hin[0:ln, 0, ch * 128:(ch + 1) * 128],
                                 w7sb[0:ln, :],
                                 start=True, stop=True)
                evict(logit[:, ch, :], po[:, :], relu=False)
            # S = sum(exp(t)); ln S via exponent-bits estimate + one Newton
            # step (only Exp needed, keeping a single act-table set).  Run in
            # two 4-chunk halves so the first half's output DMA overlaps the
            # second half's chain.
            from concourse.bass import broadcast_tensor_aps
            fi = sb.tile([128, nch], F32, name="fi", tag="fi")
            y0 = sb.tile([128, nch], F32, name="y0", tag="y0")
            ey = sb.tile([128, nch], F32, name="ey", tag="ey")
            r = sb.tile([128, nch], F32, name="r", tag="r")
            d = sb.tile([128, nch], F32, name="d", tag="d")
            s1 = sb.tile([128, nch], F32, name="s1", tag="s1")
            q = sb.tile([128, nch], F32, name="q", tag="q")
            lns = sb.tile([128, nch], F32, name="lns", tag="lns")
            osb = sb.tile([128, nch, NOUT], F32, name="osb", tag="osb")
            hc = nch // 2
            yv = y.rearrange("(c p) o -> p c o", c=nch)
            for h in range(2):
                cc = slice(h * hc, (h + 1) * hc)
                nc.scalar.activation(esb[:, cc, :], logit[:, cc, :],
                                     mybir.ActivationFunctionType.Exp)
                nc.vector.tensor_reduce(esum[:, cc], esb[:, cc, :],
                                        axis=mybir.AxisListType.X,
                                        op=mybir.AluOpType.add)
                nc.vector.tensor_copy(fi[:, cc], esum.bitcast(I32)[:, cc])
                nc.vector.tensor_scalar(y0[:, cc], fi[:, cc],
                                        EXP_BITS_BIAS, LN2_OVER_2P23,
                                        op0=mybir.AluOpType.subtract,
                                        op1=mybir.AluOpType.mult)
                nc.scalar.activation(ey[:, cc], y0[:, cc],
                                     mybir.ActivationFunctionType.Exp,
                                     scale=-1.0)
                nc.vector.tensor_tensor(r[:, cc], esum[:, cc], ey[:, cc],
                                        op=mybir.AluOpType.mult)
                nc.vector.tensor_scalar_add(d[:, cc], r[:, cc], -1.0)
                nc.vector.tensor_tensor(s1[:, cc], d[:, cc], y0[:, cc],
                                        op=mybir.AluOpType.add)
                nc.vector.scalar_tensor_tensor(q[:, cc], d[:, cc], -0.5,
                                               d[:, cc],
                                               op0=mybir.AluOpType.mult,
                                               op1=mybir.AluOpType.mult)
                nc.vector.tensor_tensor(lns[:, cc], s1[:, cc], q[:, cc],
                                        op=mybir.AluOpType.add)
                lg_ap, ln_ap = broadcast_tensor_aps(
                    logit[:, cc, :],
                    lns[:, cc].rearrange("p (c u) -> p c u", u=1))
                nc.vector.tensor_tensor(osb[:, cc, :], lg_ap, ln_ap,
                                        op=mybir.AluOpType.subtract)
                nc.sync.dma_start(out=yv[:, cc, :], in_=osb[:, cc, :])
    nc.finalize()
    return nc


# ------------------------------------------------------------------ entry point
def _prep_inputs(inputs, plans):
    np_dt = mybir.dt.np(ACT_DT)
    x = np.ascontiguousarray(np.asarray(inputs["x"], dtype=np.float32))
    shared = {}
    for i, pl in enumerate(plans):
        w1 = np.asarray(inputs[f"w1_{i + 1}"], dtype=np.float32)
        w2 = np.asarray(inputs[f"w2_{i + 1}"], dtype=np.float32)
        if isinstance(pl, LayerPlan):
            W1m, W2m = pl.build_weights(w1, w2)
            shared[f"w1c_{i}"] = np.ascontiguousarray(W1m.astype(np_dt))
            shared[f"w2c_{i}"] = np.ascontiguousarray(W2m.astype(np_dt))
        elif isinstance(pl, DensePlan):
            Wm = pl.build_weights(w1, w2)
            shared[f"wdc_{i}"] = np.ascontiguousarray(Wm.astype(np_dt))
        else:
            W = pl.build_weights(w1, w2)
            shared[f"w7c_{i}"] = np.ascontiguousarray(W.astype(np_dt))
    in_maps = []
    for c in range(NCORES):
        m = dict(shared)
        xc = x[c * BPC:(c + 1) * BPC].T.astype(np_dt)      # [3072, 1024]
        m["xT"] = np.ascontiguousarray(
            xc.reshape(plans[0].lin.ntiles, 128, BPC))
        in_maps.append(m)
    return in_maps


def _run(inputs, trace=False, **spmd_kwargs):
    plans = build_plans()
    in_maps = _prep_inputs(inputs, plans)
    nc = build_program(plans)
    res = run_bass_kernel_spmd(nc, in_maps, core_ids=list(range(NCORES)),
                               trace=trace, **spmd_kwargs)
    out = np.concatenate([r["y"] for r in res.results], axis=0)
    return out.astype(np.float32), res


def kernel(**inputs):
    out, _ = _run(inputs, trace=False)
    return out


# revision 26
# speedup vs baseline: 1.1952x; 1.1952x over previous
"""Trainium2 Bass kernel for the CIFAR10 Monarch MLP (7 monarch layers + log_softmax).

Strategy
--------
Pure data parallel over 8 NeuronCores: each core takes a 1024-row batch shard;
the ~9M-param block-diagonal weights are replicated (bf16 on device).

On-device dataflow is feature-major: activations live in SBUF as
[features (128-partition tiles), batch (free dim)], fully SBUF-resident across
all layers; only x, the weights and the final log-probs cross HBM.

Layers 1-3 keep the monarch two-GEMM structure expressed as block-sparse
matmuls over the *effective* weight matrices (butterfly permutation folded
into W1 on the host).  Layers 4-6 are fused into a single dense GEMM each
(W1eff @ W2eff), which has FEWER 128x128 tiles than the factored form at
these sizes and drops the mid evictions.  Layer 7 is fused and computed
batch-major (activation tile as the stationary matmul operand), so the
logits land as [batch, 10] in PSUM and log_softmax needs no transposes.

log_softmax avoids the Ln activation function entirely: ln(S) is computed
with a DVE exponent-extraction estimate refined by one Newton step that only
needs Exp.  Every activation function used (Copy / Relu / Exp) lives in one
activation-table set, so the program loads act tables exactly once - extra
InstLoadActFuncSet switches measurably slow *all* PE matmuls by ~20%.
"""

import os as _os

import numpy as np

import concourse.bacc as bacc_mod
import concourse.mybir as mybir
import concourse.tile as tile
from concourse.bass_utils import run_bass_kernel_spmd

# ----------------------------------------------------------------- problem dims
BATCH = 8192
IN_FEATURES = 3072
NCORES = 8
BPC = BATCH // NCORES          # 1024 batch rows per core
NOUT = 10

SHAPES = [((4, 750, 768), (4, 750, 750)),
          ((4, 500, 750), (4, 500, 500)),
          ((4, 250, 500), (4, 250, 250)),
          ((4, 125, 250), (4, 125, 125)),
          ((4, 50, 125), (4, 50, 50)),
          ((4, 25, 50), (4, 25, 25)),
          ((4, 3, 25), (4, 3, 3))]
NLAYERS = 7
NMONARCH = 3                   # layers 0..2 stay factored; 3..5 dense; 6 form-B

F32 = mybir.dt.float32
I32 = mybir.dt.int32
ACT_DT_NAME = _os.environ.get("KERNEL_MM_DT", "bf16")
ACT_DT = {"fp32": mybir.dt.float32,
          "fp32r": mybir.dt.float32r,
          "bf16": mybir.dt.bfloat16}[ACT_DT_NAME]

LN2_OVER_2P23 = float(np.log(2.0) / (1 << 23))   # 8.262958e-08
EXP_BITS_BIAS = 1065353216.0                     # bitcast(1.0f)


# ------------------------------------------------------------------ layouts
class Layout:
    """Placement of 4 feature blocks of size Sb into 128-partition tiles."""

    @classmethod
    def from_positions(cls, Sb, ntiles, feat_tile, feat_row):
        self = object.__new__(cls)
        self.Sb = Sb
        self.ntiles = ntiles
        self.feat_tile = feat_tile
        self.feat_row = feat_row
        self._finish()
        return self

    def _finish(self):
        self.valid = np.zeros(self.ntiles, np.int64)
        for k in range(4):
            for t, r in zip(self.feat_tile[k], self.feat_row[k]):
                self.valid[t] = max(self.valid[t], r + 1)
        self.grow = [self.feat_tile[k] * 128 + self.feat_row[k]
                     for k in range(4)]
        self.tiles_of_block = [sorted(set(self.feat_tile[k].tolist()))
                               for k in range(4)]


def simple_layout(Sb):
    """Blocks >= 128 rows: chunked over dedicated tiles. 65..127: one tile
    each. <= 64: packed at 32-aligned offsets."""
    if Sb >= 128:
        cpb = (Sb + 127) // 128
        ft, fr = [], []
        for k in range(4):
            i = np.arange(Sb)
            ft.append(k * cpb + i // 128)
            fr.append(i % 128)
        return Layout.from_positions(Sb, 4 * cpb, ft, fr)
    stride = ((Sb + 31) // 32) * 32
    bpt = max(1, 128 // stride)
    ntiles = (4 + bpt - 1) // bpt
    ft, fr = [], []
    for k in range(4):
        i = np.arange(Sb)
        ft.append(np.full(Sb, k // bpt, np.int64))
        fr.append((k % bpt) * stride + i)
    return Layout.from_positions(Sb, ntiles, ft, fr)


def grouped_mid_layout(R, Q):
    """Mid layout with features regrouped by input block k (R >= 125)."""
    cpb = max(1, (R + 127) // 128)
    block_rows = cpb * 128
    Gp = block_rows // 4
    ft, fr = [], []
    for l in range(4):
        rs = np.arange(R)
        ks = (4 * rs + l) // Q
        pos = np.empty(R, np.int64)
        for k in range(4):
            idx = rs[ks == k]
            assert len(idx) <= Gp
            pos[idx] = k * Gp + np.arange(len(idx))
        ft.append(l * cpb + pos // 128)
        fr.append(pos % 128)
    return Layout.from_positions(R, 4 * cpb, ft, fr)


def build_full_mats(w1_shape, w2_shape, lin, lmid, lout):
    """Shapes of the dense effective matrices (values filled on the host)."""
    return ((lin.ntiles * 128, lmid.ntiles * 128),
            (lmid.ntiles * 128, lout.ntiles * 128))


class LayerPlan:
    """Factored (monarch) layer: two block-sparse GEMMs."""

    def __init__(self, li, w1_shape, w2_shape, in_layout):
        _, Q, P = w1_shape
        _, S, R = w2_shape
        self.li, self.P, self.Q, self.R, self.S = li, P, Q, R, S
        self.lin = in_layout
        self.lmid = grouped_mid_layout(R, Q) if R >= 125 else simple_layout(R)
        self.lout = simple_layout(S)
        self.ngroups = 2 if li == 0 else 1
        self._build()

    def _build(self):
        Q, R, S = self.Q, self.R, self.S
        ks_of = [(4 * np.arange(R) + l) // Q for l in range(4)]

        need1 = {}
        for l in range(4):
            for r in range(R):
                mt = int(self.lmid.feat_tile[l][r])
                k = int(ks_of[l][r])
                need1.setdefault(mt, set()).update(self.lin.tiles_of_block[k])
        self.g1_chains = {mt: sorted(its) for mt, its in need1.items()}
        self.w1_blocks = [(mt, it) for mt in sorted(need1)
                          for it in self.g1_chains[mt]]
        self.w1_block_of = {p: i for i, p in enumerate(self.w1_blocks)}

        need2 = {}
        for l in range(4):
            for s in range(S):
                ot = int(self.lout.feat_tile[l][s])
                need2.setdefault(ot, set()).update(self.lmid.tiles_of_block[l])
        self.g2_chains = {ot: sorted(mts) for ot, mts in need2.items()}
        self.w2_blocks = [(ot, mt) for ot in sorted(need2)
                          for mt in self.g2_chains[ot]]
        self.w2_block_of = {p: i for i, p in enumerate(self.w2_blocks)}

        self.mid_tiles_of_l = [self.lmid.tiles_of_block[l] for l in range(4)]
        self.out_tiles_of_l = [self.lout.tiles_of_block[l] for l in range(4)]

    def group_lset(self, g):
        return range(4) if self.ngroups == 1 else range(2 * g, 2 * g + 2)

    def group_w1range(self, g):
        mts = {t for l in self.group_lset(g) for t in self.mid_tiles_of_l[l]}
        idxs = [i for i, (mt, _) in enumerate(self.w1_blocks) if mt in mts]
        assert idxs == list(range(idxs[0], idxs[0] + len(idxs)))
        return idxs[0], len(idxs)

    def group_w2range(self, g):
        ots = {t for l in self.group_lset(g) for t in self.out_tiles_of_l[l]}
        idxs = [i for i, (ot, _) in enumerate(self.w2_blocks) if ot in ots]
        assert idxs == list(range(idxs[0], idxs[0] + len(idxs)))
        return idxs[0], len(idxs)

    def full_mats(self, w1, w2):
        """Dense effective matrices (butterfly folded into W1)."""
        Q, R = self.Q, self.R
        W1full = np.zeros((self.lin.ntiles * 128, self.lmid.ntiles * 128),
                          np.float32)
        W2full = np.zeros((self.lmid.ntiles * 128, self.lout.ntiles * 128),
                          np.float32)
        for l in range(4):
            js = 4 * np.arange(R) + l
            ks, qs = js // Q, js % Q
            mcols = self.lmid.grow[l]
            for k in range(4):
                sel = np.where(ks == k)[0]
                if len(sel) == 0:
                    continue
                W1full[np.ix_(self.lin.grow[k], mcols[sel])] = \
                    np.ascontiguousarray(w1[k, qs[sel], :].T)
            W2full[np.ix_(self.lmid.grow[l], self.lout.grow[l])] = \
                np.ascontiguousarray(w2[l].T)
        return W1full, W2full

    def build_weights(self, w1, w2):
        """Host: gather the nonzero 128x128 tiles into [128, nblocks*128]."""
        W1full, W2full = self.full_mats(w1, w2)
        W1m = np.zeros((128, 128 * len(self.w1_blocks)), np.float32)
        for i, (mt, it) in enumerate(self.w1_blocks):
            W1m[:, i * 128:(i + 1) * 128] = \
                W1full[it * 128:(it + 1) * 128, mt * 128:(mt + 1) * 128]
        W2m = np.zeros((128, 128 * len(self.w2_blocks)), np.float32)
        for i, (ot, mt) in enumerate(self.w2_blocks):
            W2m[:, i * 128:(i + 1) * 128] = \
                W2full[mt * 128:(mt + 1) * 128, ot * 128:(ot + 1) * 128]
        return W1m, W2m


class DensePlan:
    """Fused layer: one dense GEMM over the product W1eff @ W2eff."""

    def __init__(self, li, w1_shape, w2_shape, in_layout):
        self.li = li
        self.fact = LayerPlan(li, w1_shape, w2_shape, in_layout)
        self.lin = in_layout
        self.lout = self.fact.lout
        self.blocks = [(ot, it)
                       for ot in range(self.lout.ntiles)
                       for it in range(self.lin.ntiles)]
        self.block_of = {p: i for i, p in enumerate(self.blocks)}

    def build_weights(self, w1, w2):
        W1full, W2full = self.fact.full_mats(w1, w2)
        Wd = W1full @ W2full
        Wm = np.zeros((128, 128 * len(self.blocks)), np.float32)
        for i, (ot, it) in enumerate(self.blocks):
            Wm[:, i * 128:(i + 1) * 128] = \
                Wd[it * 128:(it + 1) * 128, ot * 128:(ot + 1) * 128]
        return Wm


class FormBPlan:
    """Final layer: fused dense [in_rows x NOUT], computed batch-major with
    the activation tile as the stationary operand."""

    def __init__(self, li, w1_shape, w2_shape, in_layout):
        self.li = li
        self.fact = LayerPlan(li, w1_shape, w2_shape, in_layout)
        self.lin = in_layout
        assert self.lin.ntiles == 1
        self.in_valid = int(self.lin.valid[0])

    def build_weights(self, w1, w2):
        W1full, W2full = self.fact.full_mats(w1, w2)
        Wd = W1full @ W2full                       # [in_rows, out_grow cols]
        lout = self.fact.lout
        cols = [int(lout.grow[n // 3][n % 3]) for n in range(NOUT)]
        W = np.zeros((128, NOUT), np.float32)
        W[:self.in_valid + 0, :] = Wd[:self.in_valid, cols][: 128]
        return W


def build_plans():
    plans = []
    lin = simple_layout(SHAPES[0][0][2])
    for i, (s1, s2) in enumerate(SHAPES):
        if i < NMONARCH:
            pl = LayerPlan(i, s1, s2, lin)
        elif i < NLAYERS - 1:
            pl = DensePlan(i, s1, s2, lin)
        else:
            pl = FormBPlan(i, s1, s2, lin)
        plans.append(pl)
        lin = pl.lout if i < NLAYERS - 1 else None
    return plans


# --------------------------------------------------- numpy model of the schedule
def numpy_forward(plans, weights, xT):
    B = xT.shape[1]
    h = np.zeros((plans[0].lin.ntiles * 128, B), np.float32)
    h[:xT.shape[0]] = xT
    for pl in plans:
        if isinstance(pl, LayerPlan):
            W1m, W2m = weights[pl.li]
            mid = np.zeros((pl.lmid.ntiles * 128, B), np.float32)
            for mt, its in pl.g1_chains.items():
                V = pl.lmid.valid[mt]
                acc = np.zeros((V, B), np.float32)
                for it in its:
                    ln = pl.lin.valid[it]
                    b = pl.w1_block_of[(mt, it)]
                    acc += W1m[0:ln, b * 128:b * 128 + V].T @ \
                        h[it * 128: it * 128 + ln]
                mid[mt * 128: mt * 128 + V] = acc
            out = np.zeros((pl.lout.ntiles * 128, B), np.float32)
            for ot, mts in pl.g2_chains.items():
                V = pl.lout.valid[ot]
                acc = np.zeros((V, B), np.float32)
                for mt in mts:
                    ln = pl.lmid.valid[mt]
                    b = pl.w2_block_of[(ot, mt)]
                    acc += W2m[0:ln, b * 128:b * 128 + V].T @ \
                        mid[mt * 128: mt * 128 + ln]
                out[ot * 128: ot * 128 + V] = acc
            h = np.maximum(out, 0.0)
        elif isinstance(pl, DensePlan):
            Wm = weights[pl.li]
            out = np.zeros((pl.lout.ntiles * 128, B), np.float32)
            for ot in range(pl.lout.ntiles):
                V = pl.lout.valid[ot]
                acc = np.zeros((V, B), np.float32)
                for it in range(pl.lin.ntiles):
                    ln = pl.lin.valid[it]
                    b = pl.block_of[(ot, it)]
                    acc += Wm[0:ln, b * 128:b * 128 + V].T @ \
                        h[it * 128: it * 128 + ln]
                out[ot * 128: ot * 128 + V] = acc
            h = np.maximum(out, 0.0)
        else:
            W = weights[pl.li]                     # [128, NOUT]
            ln = pl.in_valid
            logits = h[0:ln, :].T @ W[0:ln, :]     # [B, NOUT]
            t = logits
            s = np.exp(t).sum(axis=1, keepdims=True)
            return t - np.log(s)
    raise AssertionError


# ------------------------------------------------------------------ bass program
def build_program(plans):
    nc = bacc_mod.Bacc()

    xT = nc.dram_tensor("xT", [plans[0].lin.ntiles, 128, BPC], ACT_DT,
                        kind="ExternalInput")
    wdram = []
    for i, p in enumerate(plans):
        if isinstance(p, LayerPlan):
            wdram.append((
                nc.dram_tensor(f"w1c_{i}", [128, 128 * len(p.w1_blocks)],
                               ACT_DT, kind="ExternalInput"),
                nc.dram_tensor(f"w2c_{i}", [128, 128 * len(p.w2_blocks)],
                               ACT_DT, kind="ExternalInput")))
        elif isinstance(p, DensePlan):
            wdram.append(nc.dram_tensor(f"wdc_{i}", [128, 128 * len(p.blocks)],
                                        ACT_DT, kind="ExternalInput"))
        else:
            wdram.append(nc.dram_tensor(f"w7c_{i}", [128, NOUT], ACT_DT,
                                        kind="ExternalInput"))
    y = nc.dram_tensor("y", [BPC, NOUT], F32, kind="ExternalOutput")

    with tile.TileContext(nc) as tc:
        with (
            tc.tile_pool(name="sb", bufs=1) as sb,
            tc.tile_pool(name="ps", bufs=1, space="PSUM") as ps,
        ):
            evict_flip = [0]

            def evict(dst_ap, src_ap, relu):
                e = evict_flip[0] = evict_flip[0] ^ 1
                if relu:
                    if e:
                        nc.vector.tensor_scalar_max(dst_ap, src_ap, 0.0)
                    else:
                        nc.scalar.activation(dst_ap, src_ap,
                                             mybir.ActivationFunctionType.Relu)
                else:
                    if e:
                        nc.vector.tensor_copy(dst_ap, src_ap)
                    else:
                        nc.scalar.copy(dst_ap, src_ap)

            # ---- PE p-state warm-up: the tensor engine needs ~3us of
            # continuous work to reach max clock, and the first real chains
            # otherwise run 2x slow while DMA still streams x/weights.  Burn
            # the idle startup window with dummy matmuls on a zeroed scratch
            # tile (results discarded).
            scr = sb.tile([128, 512], ACT_DT, name="scr", tag="scr")
            nc.vector.memset(scr[:, :], 0.0)
            pwarm = ps.tile([128, 512], F32, name="pwarm", tag="p7b", bufs=2)
            for _ in range(13):
                nc.tensor.matmul(pwarm[:, :], scr[0:128, 0:128], scr[:, :],
                                 start=True, stop=True)

            # ---- startup: first weight slices before/interleaved with x ----
            pl0 = plans[0]
            b1_0, b1_n = pl0.group_w1range(0)
            b2_0, b2_n = pl0.group_w2range(0)
            w1sb0 = sb.tile([128, b1_n * 128], ACT_DT, name="w1sb_0_0",
                            tag="w1")
            w2sb0 = sb.tile([128, b2_n * 128], ACT_DT, name="w2sb_0_0",
                            tag="w2")
            hin = sb.tile([128, pl0.lin.ntiles, BPC], ACT_DT,
                          name="h_in0", tag="hA")

            w1d0, w2d0 = wdram[0]
            # G1 chains of group 0 are emitted interleaved across l=0,1 (see
            # below); ship weight slices in that order, interleaved with x.
            g0_mts = []
            for a, b in zip(pl0.mid_tiles_of_l[0], pl0.mid_tiles_of_l[1]):
                g0_mts.extend((a, b))
            w1_order = []        # (block_start, block_count) per chain
            for mt in g0_mts:
                idxs = [pl0.w1_block_of[(mt, it)] - b1_0
                        for it in pl0.g1_chains[mt]]
                w1_order.append((min(idxs), len(idxs)))
            xq = [(0, 2), (2, 4), (4, 6), (6, 9), (9, 12), (12, 16),
                  (16, 20), (20, 24)]
            xi = 0

            def ship_x(n=1):
                nonlocal xi
                for _ in range(n):
                    if xi < len(xq):
                        t0, t1 = xq[xi]
                        xi += 1
                        nc.sync.dma_start(
                            out=hin[:, t0:t1, :],
                            in_=xT[t0:t1].rearrange("t p n -> p t n"))

            for ci, (s0, ns) in enumerate(w1_order):
                nc.sync.dma_start(
                    out=w1sb0[:, s0 * 128:(s0 + ns) * 128],
                    in_=w1d0[:, (b1_0 + s0) * 128:(b1_0 + s0 + ns) * 128])
                if ci == 0:
                    ship_x(2)
                elif ci % 2 == 1:
                    ship_x()
            ship_x(2)
            h2 = b2_n // 2
            nc.sync.dma_start(out=w2sb0[:, 0:h2 * 128],
                              in_=w2d0[:, b2_0 * 128:(b2_0 + h2) * 128])
            ship_x(len(xq))
            nc.sync.dma_start(out=w2sb0[:, h2 * 128:b2_n * 128],
                              in_=w2d0[:, (b2_0 + h2) * 128:(b2_0 + b2_n) * 128])

            # ---- monarch layers 0..NMONARCH-1 ----
            for li in range(NMONARCH):
                pl = plans[li]

                hnext = sb.tile([128, pl.lout.ntiles, BPC], ACT_DT,
                                name=f"h_{li + 1}",
                                tag="hB" if li % 2 == 0 else "hA")

                def g1_tile(mt, mtloc, midl, w1sb, b0, pl=pl, hin=hin):
                    V = int(pl.lmid.valid[mt])
                    its = pl.g1_chains[mt]
                    for cs in range(2):
                        c0 = cs * 512
                        pm = ps.tile([128, 512], F32, name=f"pm_{pl.li}",
                                     tag="pmid", bufs=3)
                        for j, it in enumerate(its):
                            ln = int(pl.lin.valid[it])
                            b = pl.w1_block_of[(mt, it)] - b0
                            nc.tensor.matmul(pm[0:V, :],
                                             w1sb[0:ln, b * 128:b * 128 + V],
                                             hin[0:ln, it, c0:c0 + 512],
                                             start=(j == 0),
                                             stop=(j == len(its) - 1))
                        evict(midl[0:V, mtloc, c0:c0 + 512], pm[0:V, :],
                              relu=False)

                def g2_tile(ot, mid_of, w2sb, b0, pl=pl, hnext=hnext):
                    V = int(pl.lout.valid[ot])
                    mts = pl.g2_chains[ot]
                    for cs in range(2):
                        c0 = cs * 512
                        po = ps.tile([128, 512], F32, name=f"po_{pl.li}",
                                     tag="pout", bufs=3)
                        for j, mt in enumerate(mts):
                            ln = int(pl.lmid.valid[mt])
                            b = pl.w2_block_of[(ot, mt)] - b0
                            midl, loc = mid_of[mt]
                            nc.tensor.matmul(po[0:V, :],
                                             w2sb[0:ln, b * 128:b * 128 + V],
                                             midl[0:ln, loc, c0:c0 + 512],
                                             start=(j == 0),
                                             stop=(j == len(mts) - 1))
                        evict(hnext[0:V, ot, c0:c0 + 512], po[0:V, :],
                              relu=True)

                for g in range(pl.ngroups):
                    ls = list(pl.group_lset(g))
                    b1_0, b1_n = pl.group_w1range(g)
                    b2_0, b2_n = pl.group_w2range(g)

                    if li == 0 and g == 0:
                        w1sb, w2sb = w1sb0, w2sb0
                    else:
                        w1d, w2d = wdram[li]
                        w1sb = sb.tile([128, b1_n * 128], ACT_DT,
                                       name=f"w1sb_{li}_{g}", tag="w1")
                        hn = (b1_n + 1) // 2
                        nc.sync.dma_start(
                            out=w1sb[:, 0:hn * 128],
                            in_=w1d[:, b1_0 * 128:(b1_0 + hn) * 128])
                        nc.sync.dma_start(
                            out=w1sb[:, hn * 128:b1_n * 128],
                            in_=w1d[:, (b1_0 + hn) * 128:(b1_0 + b1_n) * 128])
                        w2sb = sb.tile([128, b2_n * 128], ACT_DT,
                                       name=f"w2sb_{li}_{g}", tag="w2")
                        hn2 = (b2_n + 1) // 2
                        nc.sync.dma_start(
                            out=w2sb[:, 0:hn2 * 128],
                            in_=w2d[:, b2_0 * 128:(b2_0 + hn2) * 128])
                        nc.sync.dma_start(
                            out=w2sb[:, hn2 * 128:b2_n * 128],
                            in_=w2d[:, (b2_0 + hn2) * 128:(b2_0 + b2_n) * 128])

                    if li == 0 and g == 0:
                        # interleave G1 chains across the two l's so the
                        # earliest-arriving x tiles feed as much work as
                        # possible; G2 for both l's afterwards.
                        mid_of = {}
                        midls = {}
                        for l in ls:
                            mts_l = pl.mid_tiles_of_l[l]
                            midls[l] = sb.tile([128, len(mts_l), BPC], ACT_DT,
                                               name=f"mid_{li}_{l}",
                                               tag="midb", bufs=2)
                            for loc, mt in enumerate(mts_l):
                                mid_of[mt] = (midls[l], loc)
                        for mt in g0_mts:
                            midl, loc = mid_of[mt]
                            g1_tile(mt, loc, midl, w1sb, b1_0)
                        for l in ls:
                            for ot in pl.out_tiles_of_l[l]:
                                g2_tile(ot, mid_of, w2sb, b2_0)
                    else:
                        # per-l pipeline with one-block lookahead
                        mid_of = {}
                        pend = None
                        for l in ls:
                            mts_l = pl.mid_tiles_of_l[l]
                            midl = sb.tile([128, len(mts_l), BPC], ACT_DT,
                                           name=f"mid_{li}_{l}", tag="midb",
                                           bufs=2)
                            for loc, mt in enumerate(mts_l):
                                mid_of[mt] = (midl, loc)
                                g1_tile(mt, loc, midl, w1sb, b1_0)
                            if pend is not None:
                                for ot in pl.out_tiles_of_l[pend]:
                                    g2_tile(ot, mid_of, w2sb, b2_0)
                            pend = l
                        for ot in pl.out_tiles_of_l[pend]:
                            g2_tile(ot, mid_of, w2sb, b2_0)

                hin = hnext

            # ---- dense fused layers ----
            for li in range(NMONARCH, NLAYERS - 1):
                pl = plans[li]
                hnext = sb.tile([128, pl.lout.ntiles, BPC], ACT_DT,
                                name=f"h_{li + 1}",
                                tag="hB" if li % 2 == 0 else "hA")
                wsb = sb.tile([128, 128 * len(pl.blocks)], ACT_DT,
                              name=f"wd_{li}", tag="wd", bufs=2)
                nc.sync.dma_start(out=wsb[:, :], in_=wdram[li][:, :])
                for ot in range(pl.lout.ntiles):
                    V = int(pl.lout.valid[ot])
                    for cs in range(2):
                        c0 = cs * 512
                        po = ps.tile([128, 512], F32, name=f"po_{li}",
                                     tag="pout", bufs=3)
                        for j, it in enumerate(range(pl.lin.ntiles)):
                            ln = int(pl.lin.valid[it])
                            b = pl.block_of[(ot, it)]
                            nc.tensor.matmul(po[0:V, :],
                                             wsb[0:ln, b * 128:b * 128 + V],
                                             hin[0:ln, it, c0:c0 + 512],
                                             start=(j == 0),
                                             stop=(j == pl.lin.ntiles - 1))
                        evict(hnext[0:V, ot, c0:c0 + 512], po[0:V, :],
                              relu=True)
                hin = hnext

            # ---- final layer: batch-major logits + log_softmax ----
            pl = plans[NLAYERS - 1]
            ln = pl.in_valid
            w7sb = sb.tile([128, NOUT], ACT_DT, name="w7", tag="wd", bufs=2)
            nc.sync.dma_start(out=w7sb[:, :], in_=wdram[NLAYERS - 1][:, :])

            nch = BPC // 128
            logit = sb.tile([128, nch, NOUT], F32, name="logit", tag="logit")
            esb = sb.tile([128, nch, NOUT], F32, name="esb", tag="esb")
            esum = sb.tile([128, nch], F32, name="esum", tag="esum")
            for ch in range(nch):
                po = ps.tile([128, NOUT], F32, name="po7", tag="p7b", bufs=2)
                nc.tensor.matmul(po[:, :],
                                 hin[0:ln, 0, ch * 128:(ch + 1) * 128],
                                 w7sb[0:ln, :],
                                 start=True, stop=True)
                evict(logit[:, ch, :], po[:, :], relu=False)
            # S = sum(exp(t)); ln S via exponent-bits estimate + one Newton
            # step (only Exp needed, keeping a single act-table set).  Run in
            # two 4-chunk halves so the first half's output DMA overlaps the
            # second half's chain.
            from concourse.bass import broadcast_tensor_aps
            fi = sb.tile([128, nch], F32, name="fi", tag="fi")
            y0 = sb.tile([128, nch], F32, name="y0", tag="y0")
            ey = sb.tile([128, nch], F32, name="ey", tag="ey")
            r = sb.tile([128, nch], F32, name="r", tag="r")
            d = sb.tile([128, nch], F32, name="d", tag="d")
            s1 = sb.tile([128, nch], F32, name="s1", tag="s1")
            q = sb.tile([128, nch], F32, name="q", tag="q")
            lns = sb.tile([128, nch], F32, name="lns", tag="lns")
            osb = sb.tile([128, nch, NOUT], F32, name="osb", tag="osb")
            hc = nch // 2
            yv = y.rearrange("(c p) o -> p c o", c=nch)
            for h in range(2):
                cc = slice(h * hc, (h + 1) * hc)
                nc.scalar.activation(esb[:, cc, :], logit[:, cc, :],
                                     mybir.ActivationFunctionType.Exp)
                nc.vector.tensor_reduce(esum[:, cc], esb[:, cc, :],
                                        axis=mybir.AxisListType.X,
                                        op=mybir.AluOpType.add)
                nc.vector.tensor_copy(fi[:, cc], esum.bitcast(I32)[:, cc])
                nc.vector.tensor_scalar(y0[:, cc], fi[:, cc],
                                        EXP_BITS_BIAS, LN2_OVER_2P23,
                                        op0=mybir.AluOpType.subtract,
                                        op1=mybir.AluOpType.mult)
                nc.scalar.activation(ey[:, cc], y0[:, cc],
                                     mybir.ActivationFunctionType.Exp,
                                     scale=-1.0)
                nc.vector.tensor_tensor(r[:, cc], esum[:, cc], ey[:, cc],
                                        op=mybir.AluOpType.mult)
                nc.vector.tensor_scalar_add(d[:, cc], r[:, cc], -1.0)
                nc.vector.tensor_tensor(s1[:, cc], d[:, cc], y0[:, cc],
                                        op=mybir.AluOpType.add)
                nc.vector.scalar_tensor_tensor(q[:, cc], d[:, cc], -0.5,
                                               d[:, cc],
                                               op0=mybir.AluOpType.mult,
                                               op1=mybir.AluOpType.mult)
                nc.vector.tensor_tensor(lns[:, cc], s1[:, cc], q[:, cc],
                                        op=mybir.AluOpType.add)
                lg_ap, ln_ap = broadcast_tensor_aps(
                    logit[:, cc, :],
                    lns[:, cc].rearrange("p (c u) -> p c u", u=1))
                nc.vector.tensor_tensor(osb[:, cc, :], lg_ap, ln_ap,
                                        op=mybir.AluOpType.subtract)
                nc.sync.dma_start(out=yv[:, cc, :], in_=osb[:, cc, :])
    nc.finalize()
    return nc


# ------------------------------------------------------------------ entry point
def _prep_inputs(inputs, plans):
    np_dt = mybir.dt.np(ACT_DT)
    x = np.ascontiguousarray(np.asarray(inputs["x"], dtype=np.float32))
    shared = {}
    for i, pl in enumerate(plans):
        w1 = np.asarray(inputs[f"w1_{i + 1}"], dtype=np.float32)
        w2 = np.asarray(inputs[f"w2_{i + 1}"], dtype=np.float32)
        if isinstance(pl, LayerPlan):
            W1m, W2m = pl.build_weights(w1, w2)
            shared[f"w1c_{i}"] = np.ascontiguousarray(W1m.astype(np_dt))
            shared[f"w2c_{i}"] = np.ascontiguousarray(W2m.astype(np_dt))
        elif isinstance(pl, DensePlan):
            Wm = pl.build_weights(w1, w2)
            shared[f"wdc_{i}"] = np.ascontiguousarray(Wm.astype(np_dt))
        else:
            W = pl.build_weights(w1, w2)
            shared[f"w7c_{i}"] = np.ascontiguousarray(W.astype(np_dt))
    in_maps = []
    for c in range(NCORES):
        m = dict(shared)
        xc = x[c * BPC:(c + 1) * BPC].T.astype(np_dt)      # [3072, 1024]
        m["xT"] = np.ascontiguousarray(
            xc.reshape(plans[0].lin.ntiles, 128, BPC))
        in_maps.append(m)
    return in_maps


def _run(inputs, trace=False, **spmd_kwargs):
    plans = build_plans()
    in_maps = _prep_inputs(inputs, plans)
    nc = build_program(plans)
    res = run_bass_kernel_spmd(nc, in_maps, core_ids=list(range(NCORES)),
                               trace=trace, **spmd_kwargs)
    out = np.concatenate([r["y"] for r in res.results], axis=0)
    return out.astype(np.float32), res


def kernel(**inputs):
    out, _ = _run(inputs, trace=False)
    return out
